# revision 1
# baseline (speedup 1.0000x reference)
"""GQA attention block (RMSNorm + QKV proj + partial RoPE + causal GQA
attention + XSA correction + out proj) on 8 trn2 NeuronCores.

Sharding: 2 batches x 4 KV-groups (each core: 1 batch, 1 kv head, 4 q heads).
Each core computes a partial output (its 4 heads through its wo column slice);
the host sums the 4 partials per batch.

Layout strategy: everything transposed ([feature, token]) so the contraction
dim of every matmul lands on partitions.  fp32r matmuls for QKV/scores
(operand tiles are native float32r, produced by engine ops so walrus sees
them as rounded), bf16 for P@V and the output projection.  Softmax without
max-subtraction (logits are ~N(0,1)); causal masking zeroes invalid P
entries with gpsimd.affine_select after exp.
"""

import sys

for _p in ("/opt/trn_rl_repo", "/root/.axon_site/_ro/trn_rl_repo"):
    if _p not in sys.path:
        sys.path.append(_p)

import numpy as np
import ml_dtypes

import concourse.bass as bass
import concourse.bacc as bacc
import concourse.mybir as mybir
import concourse.tile as tile
from concourse.bass_utils import run_bass_kernel_spmd
from concourse.masks import make_identity

F32 = mybir.dt.float32
F32R = mybir.dt.float32r
BF16 = mybir.dt.bfloat16

B, T, D = 2, 2048, 2048
NH, NKV, HD = 16, 4, 128
RD = 64  # rope dims
NH_L = NH // NKV           # 4 q heads per core
EL = (NH_L + 2) * HD       # 768: q0..q3, k, v
TC = 512                   # token chunk
NTC = T // TC              # 4
DC = D // 128              # 16 contraction chunks
S128 = float(1.0 / np.sqrt(HD))
EPS = 1e-6

_CACHE = {}


def _build_nc():
    nc = bacc.Bacc("TRN2", target_bir_lowering=False, debug=False)

    xT = nc.declare_dram_parameter("xT", [D, T], F32, isOutput=False)
    wT = nc.declare_dram_parameter("wqkvT", [D, EL], F32, isOutput=False)
    woT = nc.declare_dram_parameter("woT", [NH_L * HD, D], BF16, isOutput=False)
    csP = nc.declare_dram_parameter("cs", [128, T], F32, isOutput=False)
    outp = nc.declare_dram_parameter("out", [T, D], F32, isOutput=True)

    ACT = mybir.ActivationFunctionType

    with tile.TileContext(nc) as tc:
        with (
            nc.allow_low_precision(reason="fp32r feeds matmul; tolerances ok"),
            tc.tile_pool(name="singles", bufs=1) as sg,
            tc.tile_pool(name="stream", bufs=2) as st,
            tc.tile_pool(name="ps", bufs=1, space="PSUM") as ps,
        ):
            # ---- persistent tiles ----
            w_sb = sg.tile([128, DC * EL], F32R, tag="w")
            cos_sb = sg.tile([RD, T], F32, tag="cos")
            sin_sb = sg.tile([RD, T], F32, tag="sin")
            ident = sg.tile([128, 128], F32, tag="ident")
            ones_f = sg.tile([128, 1], F32, tag="ones_f")
            ones_rf = sg.tile([1, 128], F32, tag="ones_rf")
            ones_c = sg.tile([128, 1], F32R, tag="ones_c")
            ones_r = sg.tile([1, 128], F32R, tag="ones_r")
            ones_cb = sg.tile([128, 1], BF16, tag="ones_cb")
            eps_t = sg.tile([1, 1], F32, tag="eps_t")
            qhat = [
                [sg.tile([128, TC], F32R, tag=f"qh{h}_{j}", name=f"qh{h}_{j}")
                 for j in range(NTC)]
                for h in range(NH_L)
            ]
            khat = [sg.tile([128, TC], F32R, tag=f"kh{j}", name=f"kh{j}")
                    for j in range(NTC)]
            vhat = [sg.tile([128, TC], F32, tag=f"vh{j}", name=f"vh{j}")
                    for j in range(NTC)]
            vtok = [sg.tile([128, TC], BF16, tag=f"vt{j}", name=f"vt{j}")
                    for j in range(NTC)]
            aout = [
                [sg.tile([128, TC], BF16, tag=f"ao{h}_{j}", name=f"ao{h}_{j}")
                 for j in range(NTC)]
                for h in range(NH_L)
            ]
            rvns = [sg.tile([1, TC], F32, tag=f"rvns{j}", name=f"rvns{j}")
                    for j in range(NTC)]

            nc.sync.dma_start(out=cos_sb, in_=csP[0:RD, :])
            nc.sync.dma_start(out=sin_sb, in_=csP[RD:128, :])
            make_identity(nc, ident)
            nc.vector.memset(ones_f, 1.0)
            nc.vector.memset(ones_cb, 1.0)
            nc.vector.memset(eps_t, EPS)
            nc.vector.memset(ones_rf, 1.0)
            nc.scalar.copy(ones_c, ones_f)           # rounded fp32r ones
            nc.scalar.copy(ones_r, ones_rf)
            for i in range(DC):
                wld = st.tile([128, EL], F32, tag="ld", bufs=2, name=f"wld{i}")
                nc.sync.dma_start(
                    out=wld, in_=wT[i * 128:(i + 1) * 128, :]
                )
                nc.gpsimd.tensor_copy(w_sb[:, i * EL:(i + 1) * EL], wld)

            # ========== QKV + rmsnorm scale + rope + attention, per chunk ======
            for j in range(NTC):
                js = slice(j * TC, (j + 1) * TC)
                ps_qkv = [ps.tile([128, TC], F32, tag="A", bufs=6,
                                  name=f"psqkv{j}_{_e}") for _e in range(6)]
                ps_rs = ps.tile([1, TC], F32, tag="S", bufs=2)
                for i in range(DC):
                    xt = st.tile([128, EL], F32, tag="ld", bufs=2, name=f"xt{j}_{i}")
                    nc.sync.dma_start(
                        out=xt[:, 0:TC], in_=xT[i * 128:(i + 1) * 128, js]
                    )
                    xtr = st.tile([128, TC], F32R, tag="xtr", bufs=2)
                    nc.gpsimd.tensor_copy(xtr, xt[:, 0:TC])
                    for e in range(6):
                        nc.tensor.matmul(
                            ps_qkv[e],
                            w_sb[:, i * EL + e * 128: i * EL + (e + 1) * 128],
                            xtr,
                            start=(i == 0), stop=(i == DC - 1),
                        )
                    x2r = st.tile([128, TC], F32R, tag="x2r", bufs=1)
                    nc.vector.tensor_mul(x2r, xt[:, 0:TC], xt[:, 0:TC])
                    nc.tensor.matmul(
                        ps_rs, ones_c, x2r,
                        start=(i == 0), stop=(i == DC - 1),
                    )
                # evac raw projections (rounds into fp32r tiles)
                for h in range(NH_L):
                    nc.scalar.copy(qhat[h][j], ps_qkv[h])
                nc.scalar.copy(khat[j], ps_qkv[4])
                nc.scalar.copy(vhat[j], ps_qkv[5])

                # rs = 1/sqrt(mean(x^2)+eps), broadcast across partitions
                sq = st.tile([1, TC], F32, tag="sq", bufs=1)
                nc.scalar.activation(sq, ps_rs, ACT.Sqrt, scale=1.0 / D, bias=eps_t)
                rs_t = st.tile([1, TC], F32, tag="rs_t", bufs=1)
                nc.vector.reciprocal(rs_t, sq)
                rsb = st.tile([128, TC], F32, tag="rsb", bufs=2)
                nc.gpsimd.partition_broadcast(rsb, rs_t)

                # fold rs into rope tables (in place, this chunk's columns)
                nc.vector.tensor_mul(cos_sb[:, js], cos_sb[:, js], rsb[0:RD])
                nc.vector.tensor_mul(sin_sb[:, js], sin_sb[:, js], rsb[0:RD])

                # rope + rs scaling for q heads and k (swap halves via dma)
                for th in [qhat[h][j] for h in range(NH_L)] + [khat[j]]:
                    t2s = st.tile([RD, TC], F32R, tag="t2s", bufs=2)
                    t1 = st.tile([RD, TC], F32, tag="t1", bufs=1)
                    nc.sync.dma_start(out=t2s[0:32], in_=th[32:64])
                    nc.sync.dma_start(out=t2s[32:64], in_=th[0:32])
                    nc.gpsimd.tensor_mul(t2s, t2s, sin_sb[:, js])
                    nc.vector.tensor_mul(t1, th[0:RD], cos_sb[:, js])
                    nc.vector.tensor_add(th[0:RD], t1, t2s)
                    nc.vector.tensor_mul(th[RD:128], th[RD:128], rsb[RD:128])

                # v-hat = v * rs ; 1/(sum(v^2)+eps) ; token-major v (bf16)
                nc.vector.tensor_mul(vhat[j], vhat[j], rsb)
                vsq = st.tile([128, TC], F32R, tag="vsq", bufs=1)
                nc.gpsimd.tensor_mul(vsq, vhat[j], vhat[j])
                ps_vns = ps.tile([1, TC], F32, tag="S", bufs=2)
                nc.tensor.matmul(ps_vns, ones_c, vsq, start=True, stop=True)
                vnse = st.tile([1, TC], F32, tag="vnse", bufs=1)
                nc.scalar.activation(vnse, ps_vns, ACT.Identity, bias=eps_t, scale=1.0)
                nc.vector.reciprocal(rvns[j], vnse)

                ps_vt = ps.tile([128, TC], F32, tag="A", bufs=6)
                for kk in range(TC // 128):
                    nc.tensor.transpose(
                        ps_vt[:, kk * 128:(kk + 1) * 128],
                        vhat[j][:, kk * 128:(kk + 1) * 128],
                        ident,
                    )
                nc.scalar.copy(vtok[j], ps_vt)

                # ---------------- attention for this q chunk -------------------
                for h in range(NH_L):
                    nkt = 4 * (j + 1)
                    ps_pv = ps.tile([128, TC], F32, tag="A", bufs=6)
                    ps_sum = ps.tile([1, TC], F32, tag="S", bufs=2)
                    for kt in range(nkt):
                        jk = kt // 4
                        ps_sc = ps.tile([128, TC], F32, tag="A", bufs=6)
                        nc.tensor.matmul(
                            ps_sc,
                            khat[jk][:, (kt % 4) * 128:(kt % 4 + 1) * 128],
                            qhat[h][j],
                            start=True, stop=True,
                        )
                        pT = st.tile([128, TC], BF16, tag="pT", bufs=3)
                        nc.scalar.activation(pT, ps_sc, ACT.Exp, scale=S128)
                        if kt >= 4 * j:  # diagonal-block tiles: causal zeroing
                            m = kt - 4 * j
                            nc.gpsimd.affine_select(
                                out=pT, in_=pT,
                                compare_op=mybir.AluOpType.is_ge,
                                fill=0.0,
                                base=-m * 128,
                                pattern=[[1, TC]],
                                channel_multiplier=-1,
                            )
                        nc.tensor.matmul(
                            ps_sum, ones_cb, pT,
                            start=(kt == 0), stop=(kt == nkt - 1),
                        )
                        nc.tensor.matmul(
                            ps_pv,
                            vtok[jk][:, (kt % 4) * 128:(kt % 4 + 1) * 128],
                            pT,
                            start=(kt == 0), stop=(kt == nkt - 1),
                        )
                    # normalization + XSA correction
                    inv = st.tile([1, TC], F32R, tag="inv", bufs=2)
                    nc.vector.reciprocal(inv, ps_sum)
                    pvsb = st.tile([128, TC], F32, tag="pvsb", bufs=2)
                    nc.scalar.copy(pvsb, ps_pv)
                    tu = st.tile([128, TC], F32R, tag="tu", bufs=1)
                    nc.gpsimd.tensor_mul(tu, pvsb, vhat[j])
                    ps_dot = ps.tile([1, TC], F32, tag="S", bufs=2)
                    nc.tensor.matmul(ps_dot, ones_c, tu, start=True, stop=True)
                    fu = st.tile([1, TC], F32, tag="fu", bufs=1)
                    nc.vector.tensor_mul(fu, ps_dot, rvns[j])
                    fui = st.tile([1, TC], F32R, tag="fui", bufs=1)
                    nc.vector.tensor_mul(fui, fu, inv)
                    ps_fb = ps.tile([128, TC], F32, tag="A", bufs=6)
                    nc.tensor.matmul(ps_fb, ones_r, fui, start=True, stop=True)
                    ps_ib = ps.tile([128, TC], F32, tag="A", bufs=6)
                    nc.tensor.matmul(ps_ib, ones_r, inv, start=True, stop=True)
                    m1 = st.tile([128, TC], F32, tag="mm", bufs=3, name="m1")
                    nc.vector.tensor_mul(m1, pvsb, ps_ib)
                    m2 = st.tile([128, TC], F32, tag="mm", bufs=3, name="m2")
                    nc.vector.tensor_mul(m2, vhat[j], ps_fb)
                    nc.vector.tensor_sub(aout[h][j], m1, m2)

            # ================= output projection ===========================
            for m in range(4):
                ms = slice(m * TC, (m + 1) * TC)
                wom = [st.tile([128, TC], BF16, tag="wo", bufs=4,
                               name=f"wom{m}_{_h}") for _h in range(NH_L)]
                for h in range(NH_L):
                    nc.sync.dma_start(out=wom[h], in_=woT[h * 128:(h + 1) * 128, ms])
                for tt in range(T // 128):
                    ps_o = ps.tile([128, TC], F32, tag="A", bufs=6)
                    for h in range(NH_L):
                        nc.tensor.matmul(
                            ps_o,
                            aout[h][tt // 4][:, (tt % 4) * 128:(tt % 4 + 1) * 128],
                            wom[h],
                            start=(h == 0), stop=(h == NH_L - 1),
                        )
                    osb = st.tile([128, TC], F32, tag="osb", bufs=4)
                    if tt % 2 == 0:
                        nc.scalar.copy(osb, ps_o)
                    else:
                        nc.vector.tensor_copy(osb, ps_o)
                    nc.sync.dma_start(out=outp[tt * 128:(tt + 1) * 128, ms], in_=osb)

    nc.compile()
    return nc


def _host_inputs(x, cos, sin, w_norm, wq, wk, wv, wo):
    """Build the 8 per-core input maps (host-side layout prep only)."""
    wn = w_norm.astype(np.float32)
    cosT = cos.T.astype(np.float32)                                # [64, T]
    sinT = sin.T.astype(np.float32)
    sinS = np.concatenate([-sinT[:32], sinT[32:]], axis=0)         # [64, T]
    cs = np.ascontiguousarray(np.concatenate([cosT, sinS], axis=0))  # [128, T]
    xTs = [np.ascontiguousarray(x[b].T.astype(np.float32)) for b in range(B)]
    in_maps = []
    for c in range(8):
        b, g = divmod(c, 4)
        wq_s = wq[g * NH_L * HD:(g + 1) * NH_L * HD] * wn[None, :]
        wk_s = wk[g * HD:(g + 1) * HD] * wn[None, :]
        wv_s = wv[g * HD:(g + 1) * HD] * wn[None, :]
        wqkvT = np.ascontiguousarray(
            np.concatenate([wq_s, wk_s, wv_s], axis=0).T.astype(np.float32)
        )                                                          # [D, 768]
        woT_s = np.ascontiguousarray(
            wo[:, g * NH_L * HD:(g + 1) * NH_L * HD].T
        ).astype(ml_dtypes.bfloat16)                               # [512, D]
        in_maps.append({
            "xT": xTs[b],
            "wqkvT": wqkvT,
            "woT": woT_s,
            "cs": cs,
        })
    return in_maps


def kernel(x, cos, sin, w_norm, wq, wk, wv, wo, rope_dims=64, use_xsa=1,
           **_unused):
    if "nc" not in _CACHE:
        _CACHE["nc"] = _build_nc()
    nc = _CACHE["nc"]
    in_maps = _host_inputs(
        np.asarray(x), np.asarray(cos), np.asarray(sin), np.asarray(w_norm),
        np.asarray(wq), np.asarray(wk), np.asarray(wv), np.asarray(wo),
    )
    res_obj = run_bass_kernel_spmd(nc, in_maps, list(range(8)))
    _CACHE["last"] = res_obj
    res = res_obj.results
    out = np.zeros((B, T, D), dtype=np.float32)
    for c in range(8):
        b = c // 4
        out[b] += np.asarray(res[c]["out"], dtype=np.float32)
    return out



# revision 12
# speedup vs baseline: 1.5225x; 1.5225x over previous
"""GQA attention block (RMSNorm + QKV proj + partial RoPE + causal GQA
attention + XSA correction + out proj) on 8 trn2 NeuronCores.

Sharding: 2 batches x 4 KV-groups (each core: 1 batch, 1 kv head, 4 q heads).
Each core computes a partial output (its 4 heads through its wo column slice);
the host sums the 4 partials per batch.

v2 design notes (all-bf16 dataflow):
 - host pre-casts x/wqkv/wo to bf16 and folds w_norm into the projection
   weights; no on-chip dtype casts at all.
 - rope rotate-half is turned into an adjacent-partition swap by permuting
   the first 64 rows of wq/wk (and the cos/sin tables) on the host, so the
   swap is a single DVE stream_shuffle (no sbuf-to-sbuf DMAs).
 - every reciprocal / rsqrt is computed as exp(-ln(x)) on the scalar engine;
   ln+exp live in one activation table set so there is no table thrashing.
 - causal masking uses partial-width score/sum/PV matmuls plus one
   gpsimd affine_select on the 128-wide diagonal block.
 - the PE instruction stream is ordered so QKV(j+1) / outproj(j-1) fill the
   windows where attention waits on rope/XSA, keeping the HAM clock gate
   warm (PE idle >3.4us re-throttles the PE clock 2x).
"""

import sys

for _p in ("/opt/trn_rl_repo", "/root/.axon_site/_ro/trn_rl_repo"):
    if _p not in sys.path:
        sys.path.append(_p)

import numpy as np
import ml_dtypes

import concourse.bass as bass
import concourse.bacc as bacc
import concourse.mybir as mybir
import concourse.tile as tile
from concourse.bass_utils import run_bass_kernel_spmd
from concourse.masks import make_identity

F32 = mybir.dt.float32
BF16 = mybir.dt.bfloat16

B, T, D = 2, 2048, 2048
NH, NKV, HD = 16, 4, 128
RD = 64                    # rope dims
NH_L = NH // NKV           # 4 q heads per core
EL = (NH_L + 2) * HD       # 768: q0..q3, k, v columns
TC = 512                   # token chunk
NTC = T // TC              # 4
DC = D // 128              # 16 contraction chunks
S128 = float(1.0 / np.sqrt(HD))
EPS = 1e-6

# DVE stream_shuffle mask: swap adjacent partitions within each 32-quadrant
SWAP_MASK = []
for _i in range(16):
    SWAP_MASK += [2 * _i + 1, 2 * _i]

_CACHE = {}


def _build_nc():
    nc = bacc.Bacc("TRN2", target_bir_lowering=False, debug=False)

    xT = nc.declare_dram_parameter("xT", [D, T], BF16, isOutput=False)
    wT = nc.declare_dram_parameter("wqkvT", [D, EL], BF16, isOutput=False)
    woT = nc.declare_dram_parameter("woT", [NH_L * HD, D], BF16, isOutput=False)
    csP = nc.declare_dram_parameter("cs", [128, T], BF16, isOutput=False)
    outp = nc.declare_dram_parameter("out", [T, D], F32, isOutput=True)

    ACT = mybir.ActivationFunctionType

    with tile.TileContext(nc) as tc:
        with (
            nc.allow_low_precision(reason="bf16 dataflow; rel tol 2e-2"),
            tc.tile_pool(name="singles", bufs=1) as sg,
            tc.tile_pool(name="stream", bufs=2) as st,
            tc.tile_pool(name="ps", bufs=1, space="PSUM") as ps,
        ):
            # ---------------- persistent tiles ----------------
            w_sb = sg.tile([128, DC * EL], BF16, tag="w")
            wo_sb = sg.tile([128, 16 * TC], BF16, tag="wo")
            cosP = sg.tile([RD, T], BF16, tag="cosP")
            sinP = sg.tile([RD, T], BF16, tag="sinP")
            ident = sg.tile([128, 128], BF16, tag="ident")
            ones_cb = sg.tile([128, 1], BF16, tag="ones_cb")
            eps_t = sg.tile([1, 1], F32, tag="eps_t")
            kh = [sg.tile([128, TC], BF16, tag=f"kh{j}", name=f"kh{j}")
                  for j in range(NTC)]
            vh = [sg.tile([128, TC], BF16, tag=f"vh{j}", name=f"vh{j}")
                  for j in range(NTC)]
            vt = [sg.tile([128, TC], BF16, tag=f"vt{j}", name=f"vt{j}")
                  for j in range(NTC)]
            rvns = [sg.tile([1, TC], F32, tag=f"rvns{j}", name=f"rvns{j}")
                    for j in range(NTC)]

            # x tiles live in a rotating 3-chunk window (SBUF pressure);
            # chunk j's DMAs reuse chunk j-3's buffers once qkv(j-3) is done.
            xtl = {}

            def xsl(j, i):
                return xtl[(j, i)]

            def emit_xload(j):
                js = slice(j * TC, (j + 1) * TC)
                for i in range(DC):
                    xtl[(j, i)] = st.tile([128, TC], BF16, tag="xt",
                                          bufs=3 * DC, name=f"x_{j}_{i}")
                    nc.sync.dma_start(
                        out=xtl[(j, i)], in_=xT[i * 128:(i + 1) * 128, js])

            # ---------------- initial DMAs ----------------
            nc.sync.dma_start(out=cosP, in_=csP[0:RD, :])
            nc.sync.dma_start(out=sinP, in_=csP[RD:128, :])
            # k,v weight columns first (QKV pass B starts the kernel)
            for i in range(DC):
                nc.sync.dma_start(
                    out=w_sb[:, i * EL + 4 * HD:(i + 1) * EL],
                    in_=wT[i * 128:(i + 1) * 128, 4 * HD:EL],
                )
            emit_xload(0)
            # q weight columns
            for i in range(DC):
                nc.sync.dma_start(
                    out=w_sb[:, i * EL:i * EL + 4 * HD],
                    in_=wT[i * 128:(i + 1) * 128, 0:4 * HD],
                )
            emit_xload(1)
            emit_xload(2)
            # wo tiles: (h, m) at column (h*4+m)*TC
            for h in range(NH_L):
                for m in range(4):
                    nc.sync.dma_start(
                        out=wo_sb[:, (h * 4 + m) * TC:(h * 4 + m + 1) * TC],
                        in_=woT[h * 128:(h + 1) * 128, m * TC:(m + 1) * TC],
                    )
            # chunk 3's x loads wait on chunk 0's buffers; issue them after wo
            # so the in-order sync queue doesn't hold the wo transfers back.
            emit_xload(3)

            make_identity(nc, ident)
            nc.vector.memset(ones_cb, 1.0)
            nc.vector.memset(eps_t, EPS)

            # x^2 tiles for the rms-norm sum (created one chunk ahead)
            x2 = {}
            state = {}

            def emit_x2(j):
                for i in range(DC):
                    x2[(j, i)] = st.tile([128, TC], BF16, tag="x2",
                                         bufs=DC, name=f"x2_{j}_{i}")
                    eng = nc.vector if (i % 2 == 0) else nc.gpsimd
                    eng.tensor_mul(x2[(j, i)], xsl(j, i), xsl(j, i))

            def emit_qkv(j):
                js = slice(j * TC, (j + 1) * TC)
                # ---- pass B: k, v, rs sums on PE ----
                ps_k = ps.tile([128, TC], F32, tag="big", bufs=6, name=f"psk{j}")
                ps_v = ps.tile([128, TC], F32, tag="big", bufs=6, name=f"psv{j}")
                sm_rs = ps.tile([1, TC], F32, tag="sm", bufs=2, name=f"smrs{j}")
                for i in range(DC):
                    wof = i * EL
                    nc.tensor.matmul(
                        ps_k, w_sb[:, wof + 4 * HD:wof + 5 * HD], xsl(j, i),
                        start=(i == 0), stop=(i == DC - 1))
                    nc.tensor.matmul(
                        ps_v, w_sb[:, wof + 5 * HD:wof + 6 * HD], xsl(j, i),
                        start=(i == 0), stop=(i == DC - 1))
                    nc.tensor.matmul(
                        sm_rs, ones_cb, x2[(j, i)],
                        start=(i == 0), stop=(i == DC - 1))
                # rs = exp(-0.5*ln(mean(x^2)+eps))  (scalar engine only)
                lnr = st.tile([1, TC], F32, tag="lnr", bufs=2)
                nc.scalar.activation(lnr, sm_rs, ACT.Ln, scale=1.0 / D,
                                     bias=eps_t)
                rs_t = st.tile([1, TC], BF16, tag="rs_t", bufs=2)
                nc.scalar.activation(rs_t, lnr, ACT.Exp, scale=-0.5)
                rsb = st.tile([128, TC], BF16, tag="rsb", bufs=2)
                nc.gpsimd.partition_broadcast(rsb, rs_t)

                # ---- pass A: q heads on PE ----
                ps_q = [ps.tile([128, TC], F32, tag="big", bufs=6,
                                name=f"psq{j}_{h}") for h in range(NH_L)]
                for i in range(DC):
                    wof = i * EL
                    for h in range(NH_L):
                        nc.tensor.matmul(
                            ps_q[h], w_sb[:, wof + h * HD:wof + (h + 1) * HD],
                            xsl(j, i), start=(i == 0), stop=(i == DC - 1))

                # evacuate k, v with the rms scale folded in (DVE)
                nc.vector.tensor_mul(kh[j], ps_k, rsb)
                nc.vector.tensor_mul(vh[j], ps_v, rsb)

                # v-norm sum + v token-major transpose
                vsq = st.tile([128, TC], BF16, tag="vsq", bufs=2)
                nc.gpsimd.tensor_mul(vsq, vh[j], vh[j])
                sm_vns = ps.tile([1, TC], F32, tag="sm", bufs=2, name=f"smvns{j}")
                nc.tensor.matmul(sm_vns, ones_cb, vsq, start=True, stop=True)
                ps_vt = ps.tile([128, TC], BF16, tag="big", bufs=6,
                                name=f"psvt{j}")
                for kk in range(TC // 128):
                    nc.tensor.transpose(
                        ps_vt[:, kk * 128:(kk + 1) * 128],
                        vh[j][:, kk * 128:(kk + 1) * 128],
                        ident,
                    )
                lnv = st.tile([1, TC], F32, tag="lnv", bufs=2)
                nc.scalar.activation(lnv, sm_vns, ACT.Ln, scale=1.0,
                                     bias=eps_t)
                nc.scalar.activation(rvns[j], lnv, ACT.Exp, scale=-1.0)
                nc.scalar.copy(vt[j], ps_vt)

                # evacuate q heads with rms scale (DVE)
                qhj = []
                for h in range(NH_L):
                    q = st.tile([128, TC], BF16, tag="qh", bufs=8,
                                name=f"qh{j}_{h}")
                    nc.vector.tensor_mul(q, ps_q[h], rsb)
                    qhj.append(q)
                state[("qh", j)] = qhj

                # ---- rope on k and q heads (DVE only) ----
                for th in [kh[j]] + qhj:
                    t2s = st.tile([RD, TC], BF16, tag="t2s", bufs=2)
                    nc.vector.stream_shuffle(t2s, th[0:RD], SWAP_MASK)
                    nc.vector.tensor_mul(th[0:RD], th[0:RD], cosP[:, js])
                    nc.vector.tensor_mul(t2s, t2s, sinP[:, js])
                    nc.vector.tensor_add(th[0:RD], th[0:RD], t2s)

            def emit_attn_pair(j, pair):
                heads = (0, 1) if pair == 0 else (2, 3)
                qhj = state[("qh", j)]
                nkt = 4 * (j + 1)
                ps_pv = {
                    h: ps.tile([128, TC], F32, tag="big", bufs=6,
                               name=f"pspv{j}_{h}")
                    for h in heads
                }
                sm_sum = ps.tile([64, TC], F32, tag="sm", bufs=2,
                                 name=f"smsum{j}_{pair}")
                pT = {}

                def tile_geom(kt):
                    cs = 128 * (kt - 4 * j) if kt >= 4 * j else 0
                    return cs, TC - cs

                def emit_sum_pv(kt):
                    cs, _w = tile_geom(kt)
                    jk, kk = divmod(kt, 4)
                    for h in heads:
                        p = pT[(h, kt)][:, cs:TC]
                        nc.tensor.matmul(
                            sm_sum[32 * (h % 2):32 * (h % 2) + 1, cs:TC],
                            ones_cb, p,
                            start=(kt == 0), stop=(kt == nkt - 1))
                        nc.tensor.matmul(
                            ps_pv[h][:, cs:TC],
                            vt[jk][:, kk * 128:(kk + 1) * 128], p,
                            start=(kt == 0), stop=(kt == nkt - 1))

                for kt in range(nkt):
                    cs, _w = tile_geom(kt)
                    jk, kk = divmod(kt, 4)
                    for h in heads:
                        ps_sc = ps.tile([128, TC], F32, tag="big", bufs=6,
                                        name=f"pssc{j}_{h}_{kt}")
                        nc.tensor.matmul(
                            ps_sc[:, cs:TC],
                            kh[jk][:, kk * 128:(kk + 1) * 128],
                            qhj[h][:, cs:TC],
                            start=True, stop=True)
                        pt = st.tile([128, TC], BF16, tag="pT", bufs=6,
                                     name=f"pt{j}_{h}_{kt}")
                        nc.scalar.activation(
                            pt[:, cs:TC], ps_sc[:, cs:TC], ACT.Exp, scale=S128)
                        if kt >= 4 * j:
                            # zero strictly-above-diagonal entries in the
                            # 128-wide diagonal block: keep col >= partition
                            nc.gpsimd.affine_select(
                                out=pt[:, cs:cs + 128], in_=pt[:, cs:cs + 128],
                                compare_op=mybir.AluOpType.is_ge,
                                fill=0.0, base=0,
                                pattern=[[1, 128]],
                                channel_multiplier=-1,
                            )
                        pT[(h, kt)] = pt
                    if kt > 0:
                        emit_sum_pv(kt - 1)
                emit_sum_pv(nkt - 1)
                state[(j, pair)] = (ps_pv, sm_sum)

            def emit_xsa_pre(j, pair):
                heads = (0, 1) if pair == 0 else (2, 3)
                ps_pv, sm_sum = state[(j, pair)]
                pvsb, tu, inv = {}, {}, {}
                for h in heads:
                    pvsb[h] = st.tile([128, TC], BF16, tag="pvsb", bufs=4,
                                      name=f"pvsb{j}_{h}")
                    nc.scalar.copy(pvsb[h], ps_pv[h])
                    lnS = st.tile([1, TC], F32, tag="lnS", bufs=2)
                    nc.scalar.activation(
                        lnS, sm_sum[32 * (h % 2):32 * (h % 2) + 1, :], ACT.Ln)
                    inv[h] = st.tile([1, TC], BF16, tag="inv", bufs=4,
                                     name=f"inv{j}_{h}")
                    nc.scalar.activation(inv[h], lnS, ACT.Exp, scale=-1.0)
                    tu[h] = st.tile([128, TC], BF16, tag="tu", bufs=4,
                                    name=f"tu{j}_{h}")
                    nc.vector.tensor_mul(tu[h], pvsb[h], vh[j])
                state[(j, pair, "pre")] = (pvsb, tu, inv)

            def emit_dots(j, pair):
                heads = (0, 1) if pair == 0 else (2, 3)
                _, tu, _ = state[(j, pair, "pre")]
                sm_dot = ps.tile([64, TC], F32, tag="sm", bufs=2,
                                 name=f"smdot{j}_{pair}")
                for h in heads:
                    nc.tensor.matmul(
                        sm_dot[32 * (h % 2):32 * (h % 2) + 1, :],
                        ones_cb, tu[h], start=True, stop=True)
                state[(j, pair, "dot")] = sm_dot

            def emit_xsa_post(j, pair):
                heads = (0, 1) if pair == 0 else (2, 3)
                pvsb, tu, inv = state[(j, pair, "pre")]
                sm_dot = state[(j, pair, "dot")]
                for h in heads:
                    t1 = st.tile([1, TC], F32, tag="t1", bufs=2)
                    nc.vector.tensor_mul(
                        t1, sm_dot[32 * (h % 2):32 * (h % 2) + 1, :], rvns[j])
                    fui = st.tile([1, TC], BF16, tag="fui", bufs=2)
                    nc.vector.tensor_mul(fui, t1, inv[h])
                    invB = st.tile([128, TC], BF16, tag="invB", bufs=2)
                    nc.gpsimd.partition_broadcast(invB, inv[h])
                    fuiB = st.tile([128, TC], BF16, tag="fuiB", bufs=2)
                    nc.gpsimd.partition_broadcast(fuiB, fui)
                    m1 = st.tile([128, TC], BF16, tag="m1", bufs=2)
                    nc.vector.tensor_mul(m1, pvsb[h], invB)
                    m2 = st.tile([128, TC], BF16, tag="m2", bufs=2)
                    nc.vector.tensor_mul(m2, vh[j], fuiB)
                    aot = st.tile([128, TC], BF16, tag="ao", bufs=6,
                                  name=f"ao{j}_{h}")
                    nc.vector.tensor_sub(aot, m1, m2)
                    state[("ao", j, h)] = aot

            def emit_outproj(j):
                for m in range(4):
                    ms = slice(m * TC, (m + 1) * TC)
                    for tt in range(4):
                        ps_o = ps.tile([128, TC], F32, tag="big", bufs=6,
                                       name=f"pso{j}_{m}_{tt}")
                        for h in range(NH_L):
                            nc.tensor.matmul(
                                ps_o,
                                state[("ao", j, h)][:, tt * 128:(tt + 1) * 128],
                                wo_sb[:, (h * 4 + m) * TC:(h * 4 + m + 1) * TC],
                                start=(h == 0), stop=(h == NH_L - 1))
                        osb = st.tile([128, TC], F32, tag="osb", bufs=3)
                        if tt % 2 == 0:
                            nc.scalar.copy(osb, ps_o)
                        else:
                            nc.vector.tensor_copy(osb, ps_o)
                        nc.sync.dma_start(
                            out=outp[(j * 4 + tt) * 128:(j * 4 + tt + 1) * 128,
                                     ms],
                            in_=osb)

            # ================= the schedule =================
            # PE stream: QKV(0) | attnA(0) attnB(0) dotsA(0) | QKV(1) dotsB(0)
            #            outproj(0) | attnA(1) ... so rope(j+1)/XSA(j) on the
            #            other engines always overlap PE matmul phases.
            emit_x2(0)
            emit_qkv(0)
            for j in range(NTC):
                emit_attn_pair(j, 0)
                emit_xsa_pre(j, 0)
                emit_attn_pair(j, 1)
                emit_xsa_pre(j, 1)
                emit_dots(j, 0)
                emit_xsa_post(j, 0)
                if j + 1 < NTC:
                    emit_x2(j + 1)
                    emit_qkv(j + 1)
                emit_dots(j, 1)
                emit_xsa_post(j, 1)
                emit_outproj(j)

    nc.compile()
    return nc


def _host_inputs(x, cos, sin, w_norm, wq, wk, wv, wo):
    """Build the 8 per-core input maps (host-side layout prep only)."""
    wn = w_norm.astype(np.float32)
    # rope interleave permutation: pair (i, i+32) -> positions (2i, 2i+1)
    p64 = np.empty(64, np.int64)
    p64[0::2] = np.arange(32)
    p64[1::2] = np.arange(32, 64)
    perm = np.concatenate([p64, np.arange(64, HD)])

    cosT = cos.T.astype(np.float32)        # [64, T], cos[i] == cos[i+32]
    sinT = sin.T.astype(np.float32)
    cosP = cosT[p64]
    sinP = np.empty_like(cosP)
    sinP[0::2] = -sinT[:32]
    sinP[1::2] = sinT[:32]
    cs = np.ascontiguousarray(
        np.concatenate([cosP, sinP], axis=0)).astype(ml_dtypes.bfloat16)

    xTs = [np.ascontiguousarray(x[b].T).astype(ml_dtypes.bfloat16)
           for b in range(B)]
    in_maps = []
    for c in range(8):
        b, g = divmod(c, 4)
        wq_s = (wq[g * NH_L * HD:(g + 1) * NH_L * HD] * wn[None, :]).reshape(
            NH_L, HD, D)[:, perm, :].reshape(NH_L * HD, D)
        wk_s = (wk[g * HD:(g + 1) * HD] * wn[None, :])[perm]
        wv_s = wv[g * HD:(g + 1) * HD] * wn[None, :]
        wqkvT = np.ascontiguousarray(
            np.concatenate([wq_s, wk_s, wv_s], axis=0).T
        ).astype(ml_dtypes.bfloat16)                       # [D, 768]
        woT_s = np.ascontiguousarray(
            wo[:, g * NH_L * HD:(g + 1) * NH_L * HD].T
        ).astype(ml_dtypes.bfloat16)                       # [512, D]
        in_maps.append({
            "xT": xTs[b],
            "wqkvT": wqkvT,
            "woT": woT_s,
            "cs": cs,
        })
    return in_maps


def kernel(x, cos, sin, w_norm, wq, wk, wv, wo, rope_dims=64, use_xsa=1,
           **_unused):
    if "nc" not in _CACHE:
        _CACHE["nc"] = _build_nc()
    nc = _CACHE["nc"]
    in_maps = _host_inputs(
        np.asarray(x), np.asarray(cos), np.asarray(sin), np.asarray(w_norm),
        np.asarray(wq), np.asarray(wk), np.asarray(wv), np.asarray(wo),
    )
    res_obj = run_bass_kernel_spmd(nc, in_maps, list(range(8)))
    _CACHE["last"] = res_obj
    res = res_obj.results
    out = np.zeros((B, T, D), dtype=np.float32)
    for c in range(8):
        b = c // 4
        out[b] += np.asarray(res[c]["out"], dtype=np.float32)
    return out


# revision 18
# speedup vs baseline: 1.8702x; 1.2284x over previous
"""GQA attention block (RMSNorm + QKV proj + partial RoPE + causal GQA
attention + XSA correction + out proj) on 8 trn2 NeuronCores.

Sharding: 2 batches x 4 KV-groups (each core: 1 batch, 1 kv head, 4 q heads).
Each core computes a partial output (its 4 heads through its wo column slice);
the host sums the 4 partials per batch.

v2 design notes (all-bf16 dataflow):
 - host pre-casts x/wqkv/wo to bf16 and folds w_norm into the projection
   weights; no on-chip dtype casts at all.
 - rope rotate-half is turned into an adjacent-partition swap by permuting
   the first 64 rows of wq/wk (and the cos/sin tables) on the host, so the
   swap is a single DVE stream_shuffle (no sbuf-to-sbuf DMAs).
 - every reciprocal / rsqrt is computed as exp(-ln(x)) on the scalar engine;
   ln+exp live in one activation table set so there is no table thrashing.
 - causal masking uses partial-width score/sum/PV matmuls plus one
   gpsimd affine_select on the 128-wide diagonal block.
 - the PE instruction stream is ordered so QKV(j+1) / outproj(j-1) fill the
   windows where attention waits on rope/XSA, keeping the HAM clock gate
   warm (PE idle >3.4us re-throttles the PE clock 2x).
"""

import sys

for _p in ("/opt/trn_rl_repo", "/root/.axon_site/_ro/trn_rl_repo"):
    if _p not in sys.path:
        sys.path.append(_p)

import numpy as np
import ml_dtypes

import concourse.bass as bass
import concourse.bacc as bacc
import concourse.mybir as mybir
import concourse.tile as tile
from concourse import hw_specs as _hw_specs
from concourse.bass_utils import run_bass_kernel_spmd
from concourse.masks import make_identity

# The activation-table chooser maps Ln -> "natural_log" and Exp ->
# "exp_and_others", so a kernel alternating ln/exp reloads the table RAMs
# (~2.7us) on every switch.  Both functions live together in
# "natural_log_exp_and_others"; restrict them to that set so exactly one
# table load is ever emitted.
_ORIG_GAT = _hw_specs.get_activation_tables


def _gat_combined(arch):
    tabs = _ORIG_GAT(arch)
    keep = "natural_log_exp_and_others"
    if keep in tabs:
        ln = mybir.ActivationFunctionType.Ln
        ex = mybir.ActivationFunctionType.Exp
        for nm, fns in tabs.items():
            if nm != keep:
                fns.discard(ln)
                fns.discard(ex)
    return tabs


_hw_specs.get_activation_tables = _gat_combined
bacc.get_activation_tables = _gat_combined

F32 = mybir.dt.float32
BF16 = mybir.dt.bfloat16

B, T, D = 2, 2048, 2048
NH, NKV, HD = 16, 4, 128
RD = 64                    # rope dims
NH_L = NH // NKV           # 4 q heads per core
EL = (NH_L + 2) * HD       # 768: q0..q3, k, v columns
TC = 512                   # token chunk
NTC = T // TC              # 4
DC = D // 128              # 16 contraction chunks
S128 = float(1.0 / np.sqrt(HD))
EPS = 1e-6

# DVE stream_shuffle mask: swap adjacent partitions within each 32-quadrant
SWAP_MASK = []
for _i in range(16):
    SWAP_MASK += [2 * _i + 1, 2 * _i]

_CACHE = {}


def _build_nc():
    nc = bacc.Bacc("TRN2", target_bir_lowering=False, debug=False)

    xT = nc.declare_dram_parameter("xT", [D, T], BF16, isOutput=False)
    wT = nc.declare_dram_parameter("wqkvT", [D, EL], BF16, isOutput=False)
    woT = nc.declare_dram_parameter("woT", [NH_L * HD, D], BF16, isOutput=False)
    csP = nc.declare_dram_parameter("cs", [128, T], BF16, isOutput=False)
    outp = nc.declare_dram_parameter("out", [T, D], F32, isOutput=True)

    ACT = mybir.ActivationFunctionType

    with tile.TileContext(nc) as tc:
        with (
            nc.allow_low_precision(reason="bf16 dataflow; rel tol 2e-2"),
            tc.tile_pool(name="singles", bufs=1) as sg,
            tc.tile_pool(name="stream", bufs=2) as st,
            tc.tile_pool(name="ps", bufs=1, space="PSUM") as ps,
        ):
            # ---------------- persistent tiles ----------------
            w_sb = sg.tile([128, DC * EL], BF16, tag="w")
            wo_sb = sg.tile([128, 16 * TC], BF16, tag="wo")
            cosP = sg.tile([RD, T], BF16, tag="cosP")
            sinP = sg.tile([RD, T], BF16, tag="sinP")
            ident = sg.tile([128, 128], BF16, tag="ident")
            ones_cb = sg.tile([128, 1], BF16, tag="ones_cb")
            eps_t = sg.tile([1, 1], F32, tag="eps_t")
            kh = [sg.tile([128, TC], BF16, tag=f"kh{j}", name=f"kh{j}")
                  for j in range(NTC)]
            vh = [sg.tile([128, TC], BF16, tag=f"vh{j}", name=f"vh{j}")
                  for j in range(NTC)]
            vt = [sg.tile([128, TC], BF16, tag=f"vt{j}", name=f"vt{j}")
                  for j in range(NTC)]
            rvns = [sg.tile([1, TC], F32, tag=f"rvns{j}", name=f"rvns{j}")
                    for j in range(NTC)]

            # x tiles live in a rotating 3-chunk window (SBUF pressure);
            # chunk j's DMAs reuse chunk j-3's buffers once qkv(j-3) is done.
            xtl = {}

            def xsl(j, i):
                return xtl[(j, i)]

            def emit_xload(j):
                js = slice(j * TC, (j + 1) * TC)
                for i in range(DC):
                    xtl[(j, i)] = st.tile([128, TC], BF16, tag="xt",
                                          bufs=3 * DC, name=f"x_{j}_{i}")
                    nc.sync.dma_start(
                        out=xtl[(j, i)], in_=xT[i * 128:(i + 1) * 128, js])

            # ---------------- initial DMAs ----------------
            nc.sync.dma_start(out=cosP, in_=csP[0:RD, :])
            nc.sync.dma_start(out=sinP, in_=csP[RD:128, :])
            # interleave k/v weight columns with x chunk 0 so the i-th
            # pass-B matmul can start as soon as its own tiles land
            for i in range(DC):
                nc.sync.dma_start(
                    out=w_sb[:, i * EL + 4 * HD:(i + 1) * EL],
                    in_=wT[i * 128:(i + 1) * 128, 4 * HD:EL],
                )
                xtl[(0, i)] = st.tile([128, TC], BF16, tag="xt",
                                      bufs=3 * DC, name=f"x_0_{i}")
                nc.sync.dma_start(
                    out=xtl[(0, i)], in_=xT[i * 128:(i + 1) * 128, 0:TC])
            # q weight columns
            for i in range(DC):
                nc.sync.dma_start(
                    out=w_sb[:, i * EL:i * EL + 4 * HD],
                    in_=wT[i * 128:(i + 1) * 128, 0:4 * HD],
                )
            emit_xload(1)
            emit_xload(2)
            # wo tiles: (h, m) at column (h*4+m)*TC
            for h in range(NH_L):
                for m in range(4):
                    nc.sync.dma_start(
                        out=wo_sb[:, (h * 4 + m) * TC:(h * 4 + m + 1) * TC],
                        in_=woT[h * 128:(h + 1) * 128, m * TC:(m + 1) * TC],
                    )
            # chunk 3's x loads wait on chunk 0's buffers; issue them after wo
            # so the in-order sync queue doesn't hold the wo transfers back.
            emit_xload(3)

            make_identity(nc, ident)
            nc.vector.memset(ones_cb, 1.0)
            nc.vector.memset(eps_t, EPS)

            # x^2 tiles for the rms-norm sum (created one chunk ahead)
            x2 = {}
            state = {}

            def emit_x2(j):
                for i in range(DC):
                    x2[(j, i)] = st.tile([128, TC], BF16, tag="x2",
                                         bufs=DC, name=f"x2_{j}_{i}")
                    nc.vector.tensor_mul(x2[(j, i)], xsl(j, i), xsl(j, i))

            def emit_qkv(j):
                js = slice(j * TC, (j + 1) * TC)
                # ---- pass B: k, v, rs sums on PE ----
                ps_k = ps.tile([128, TC], F32, tag="big", bufs=6, name=f"psk{j}")
                ps_v = ps.tile([128, TC], F32, tag="big", bufs=6, name=f"psv{j}")
                sm_rs = ps.tile([1, TC], F32, tag="sm", bufs=2, name=f"smrs{j}")
                for i in range(DC):
                    wof = i * EL
                    nc.tensor.matmul(
                        ps_k, w_sb[:, wof + 4 * HD:wof + 5 * HD], xsl(j, i),
                        start=(i == 0), stop=(i == DC - 1))
                    nc.tensor.matmul(
                        ps_v, w_sb[:, wof + 5 * HD:wof + 6 * HD], xsl(j, i),
                        start=(i == 0), stop=(i == DC - 1))
                    nc.tensor.matmul(
                        sm_rs, ones_cb, x2[(j, i)],
                        start=(i == 0), stop=(i == DC - 1))
                # rs = exp(-0.5*ln(mean(x^2)+eps))  (scalar engine only)
                lnr = st.tile([1, TC], F32, tag="lnr", bufs=2)
                nc.scalar.activation(lnr, sm_rs, ACT.Ln, scale=1.0 / D,
                                     bias=eps_t)
                rs_t = st.tile([1, TC], BF16, tag="rs_t", bufs=2)
                nc.scalar.activation(rs_t, lnr, ACT.Exp, scale=-0.5)
                rsb = st.tile([128, TC], BF16, tag="rsb", bufs=2)
                nc.gpsimd.partition_broadcast(rsb, rs_t)

                # ---- pass A: q heads on PE ----
                ps_q = [ps.tile([128, TC], F32, tag="big", bufs=6,
                                name=f"psq{j}_{h}") for h in range(NH_L)]
                for i in range(DC):
                    wof = i * EL
                    for h in range(NH_L):
                        nc.tensor.matmul(
                            ps_q[h], w_sb[:, wof + h * HD:wof + (h + 1) * HD],
                            xsl(j, i), start=(i == 0), stop=(i == DC - 1))

                # evacuate k, v with the rms scale folded in (DVE)
                nc.vector.tensor_mul(kh[j], ps_k, rsb)
                nc.vector.tensor_mul(vh[j], ps_v, rsb)
                vsq = st.tile([128, TC], BF16, tag="vsq", bufs=2,
                              name=f"vsq{j}")
                nc.vector.tensor_mul(vsq, vh[j], vh[j])
                state[("vsq", j)] = vsq

                # evacuate q heads with rms scale (DVE)
                qhj = []
                for h in range(NH_L):
                    q = st.tile([128, TC], BF16, tag="qh", bufs=8,
                                name=f"qh{j}_{h}")
                    nc.vector.tensor_mul(q, ps_q[h], rsb)
                    qhj.append(q)
                state[("qh", j)] = qhj

                # ---- rope on k and q heads (DVE only) ----
                for th in [kh[j]] + qhj:
                    t2s = st.tile([RD, TC], BF16, tag="t2s", bufs=2)
                    nc.vector.stream_shuffle(t2s, th[0:RD], SWAP_MASK)
                    nc.vector.tensor_mul(th[0:RD], th[0:RD], cosP[:, js])
                    nc.vector.tensor_mul(t2s, t2s, sinP[:, js])
                    nc.vector.tensor_add(th[0:RD], th[0:RD], t2s)

            def emit_vnsvt(j):
                # v-norm sum + v token-major transpose; scheduled as its own
                # PE phase late enough that vsq(j) (DVE) is long done, so
                # these in-order PE mms never block the queue.
                vsq = state[("vsq", j)]
                ps_vt = ps.tile([128, TC], BF16, tag="big", bufs=6,
                                name=f"psvt{j}")
                for kk in range(TC // 128):
                    nc.tensor.transpose(
                        ps_vt[:, kk * 128:(kk + 1) * 128],
                        vh[j][:, kk * 128:(kk + 1) * 128],
                        ident,
                    )
                sm_vns = ps.tile([1, TC], F32, tag="sm", bufs=2,
                                 name=f"smvns{j}")
                nc.tensor.matmul(sm_vns, ones_cb, vsq, start=True, stop=True)
                lnv = st.tile([1, TC], F32, tag="lnv", bufs=2)
                nc.scalar.activation(lnv, sm_vns, ACT.Ln, scale=1.0,
                                     bias=eps_t)
                nc.scalar.activation(rvns[j], lnv, ACT.Exp, scale=-1.0)
                nc.scalar.copy(vt[j], ps_vt)

            def emit_attn_pair(j, pair):
                heads = (0, 1) if pair == 0 else (2, 3)
                qhj = state[("qh", j)]
                nkt = 4 * (j + 1)
                ps_pv = {
                    h: ps.tile([128, TC], F32, tag="big", bufs=6,
                               name=f"pspv{j}_{h}")
                    for h in heads
                }
                sm_sum = ps.tile([64, TC], F32, tag="sm", bufs=2,
                                 name=f"smsum{j}_{pair}")
                pT = {}

                def tile_geom(kt):
                    cs = 128 * (kt - 4 * j) if kt >= 4 * j else 0
                    return cs, TC - cs

                def emit_sum_pv(kt):
                    cs, _w = tile_geom(kt)
                    jk, kk = divmod(kt, 4)
                    for h in heads:
                        p = pT[(h, kt)][:, cs:TC]
                        nc.tensor.matmul(
                            sm_sum[32 * (h % 2):32 * (h % 2) + 1, cs:TC],
                            ones_cb, p,
                            start=(kt == 0), stop=(kt == nkt - 1))
                        nc.tensor.matmul(
                            ps_pv[h][:, cs:TC],
                            vt[jk][:, kk * 128:(kk + 1) * 128], p,
                            start=(kt == 0), stop=(kt == nkt - 1))

                for kt in range(nkt):
                    cs, _w = tile_geom(kt)
                    jk, kk = divmod(kt, 4)
                    for h in heads:
                        ps_sc = ps.tile([128, TC], F32, tag="big", bufs=6,
                                        name=f"pssc{j}_{h}_{kt}")
                        nc.tensor.matmul(
                            ps_sc[:, cs:TC],
                            kh[jk][:, kk * 128:(kk + 1) * 128],
                            qhj[h][:, cs:TC],
                            start=True, stop=True)
                        pt = st.tile([128, TC], BF16, tag="pT", bufs=6,
                                     name=f"pt{j}_{h}_{kt}")
                        nc.scalar.activation(
                            pt[:, cs:TC], ps_sc[:, cs:TC], ACT.Exp, scale=S128)
                        if kt >= 4 * j:
                            # zero strictly-above-diagonal entries in the
                            # 128-wide diagonal block: keep col >= partition
                            nc.gpsimd.affine_select(
                                out=pt[:, cs:cs + 128], in_=pt[:, cs:cs + 128],
                                compare_op=mybir.AluOpType.is_ge,
                                fill=0.0, base=0,
                                pattern=[[1, 128]],
                                channel_multiplier=-1,
                            )
                        pT[(h, kt)] = pt
                    if kt > 0:
                        emit_sum_pv(kt - 1)
                emit_sum_pv(nkt - 1)
                state[(j, pair)] = (ps_pv, sm_sum)

            def emit_xsa_pre(j, pair):
                heads = (0, 1) if pair == 0 else (2, 3)
                ps_pv, sm_sum = state[(j, pair)]
                pvsb, tu, inv = {}, {}, {}
                for h in heads:
                    pvsb[h] = st.tile([128, TC], BF16, tag="pvsb", bufs=4,
                                      name=f"pvsb{j}_{h}")
                    nc.scalar.copy(pvsb[h], ps_pv[h])
                    lnS = st.tile([1, TC], F32, tag="lnS", bufs=2)
                    nc.scalar.activation(
                        lnS, sm_sum[32 * (h % 2):32 * (h % 2) + 1, :], ACT.Ln)
                    inv[h] = st.tile([1, TC], BF16, tag="inv", bufs=4,
                                     name=f"inv{j}_{h}")
                    nc.scalar.activation(inv[h], lnS, ACT.Exp, scale=-1.0)
                    tu[h] = st.tile([128, TC], BF16, tag="tu", bufs=4,
                                    name=f"tu{j}_{h}")
                    nc.vector.tensor_mul(tu[h], pvsb[h], vh[j])
                state[(j, pair, "pre")] = (pvsb, tu, inv)

            def emit_dots(j, pair):
                heads = (0, 1) if pair == 0 else (2, 3)
                _, tu, _ = state[(j, pair, "pre")]
                sm_dot = ps.tile([64, TC], F32, tag="sm", bufs=2,
                                 name=f"smdot{j}_{pair}")
                for h in heads:
                    nc.tensor.matmul(
                        sm_dot[32 * (h % 2):32 * (h % 2) + 1, :],
                        ones_cb, tu[h], start=True, stop=True)
                state[(j, pair, "dot")] = sm_dot

            def emit_xsa_post(j, pair):
                heads = (0, 1) if pair == 0 else (2, 3)
                pvsb, tu, inv = state[(j, pair, "pre")]
                sm_dot = state[(j, pair, "dot")]
                for h in heads:
                    t1 = st.tile([1, TC], F32, tag="t1", bufs=2)
                    nc.vector.tensor_mul(
                        t1, sm_dot[32 * (h % 2):32 * (h % 2) + 1, :], rvns[j])
                    fui = st.tile([1, TC], BF16, tag="fui", bufs=2)
                    nc.vector.tensor_mul(fui, t1, inv[h])
                    invB = st.tile([128, TC], BF16, tag="invB", bufs=2)
                    nc.gpsimd.partition_broadcast(invB, inv[h])
                    fuiB = st.tile([128, TC], BF16, tag="fuiB", bufs=2)
                    nc.gpsimd.partition_broadcast(fuiB, fui)
                    m1 = st.tile([128, TC], BF16, tag="m1", bufs=2)
                    nc.vector.tensor_mul(m1, pvsb[h], invB)
                    m2 = st.tile([128, TC], BF16, tag="m2", bufs=2)
                    nc.vector.tensor_mul(m2, vh[j], fuiB)
                    aot = st.tile([128, TC], BF16, tag="ao", bufs=6,
                                  name=f"ao{j}_{h}")
                    nc.vector.tensor_sub(aot, m1, m2)
                    state[("ao", j, h)] = aot

            def emit_outproj(j):
                for m in range(4):
                    ms = slice(m * TC, (m + 1) * TC)
                    for tt in range(4):
                        ps_o = ps.tile([128, TC], F32, tag="big", bufs=6,
                                       name=f"pso{j}_{m}_{tt}")
                        for h in range(NH_L):
                            nc.tensor.matmul(
                                ps_o,
                                state[("ao", j, h)][:, tt * 128:(tt + 1) * 128],
                                wo_sb[:, (h * 4 + m) * TC:(h * 4 + m + 1) * TC],
                                start=(h == 0), stop=(h == NH_L - 1))
                        osb = st.tile([128, TC], F32, tag="osb", bufs=3)
                        if tt % 2 == 0:
                            nc.scalar.copy(osb, ps_o)
                        else:
                            nc.vector.tensor_copy(osb, ps_o)
                        nc.sync.dma_start(
                            out=outp[(j * 4 + tt) * 128:(j * 4 + tt + 1) * 128,
                                     ms],
                            in_=osb)

            # ================= the schedule =================
            # PE stream: QKV(0) | attnA(0) attnB(0) dotsA(0) | QKV(1) dotsB(0)
            #            outproj(0) | attnA(1) ... so rope(j+1)/XSA(j) on the
            #            other engines always overlap PE matmul phases.
            emit_x2(0)
            emit_qkv(0)
            emit_vnsvt(0)
            for j in range(NTC):
                emit_attn_pair(j, 0)
                emit_xsa_pre(j, 0)
                emit_attn_pair(j, 1)
                emit_xsa_pre(j, 1)
                emit_dots(j, 0)
                emit_xsa_post(j, 0)
                if j + 1 < NTC:
                    emit_x2(j + 1)
                    emit_qkv(j + 1)
                emit_dots(j, 1)
                emit_xsa_post(j, 1)
                emit_outproj(j)
                if j + 1 < NTC:
                    emit_vnsvt(j + 1)

    nc.compile()
    return nc


def _host_inputs(x, cos, sin, w_norm, wq, wk, wv, wo):
    """Build the 8 per-core input maps (host-side layout prep only)."""
    wn = w_norm.astype(np.float32)
    # rope interleave permutation: pair (i, i+32) -> positions (2i, 2i+1)
    p64 = np.empty(64, np.int64)
    p64[0::2] = np.arange(32)
    p64[1::2] = np.arange(32, 64)
    perm = np.concatenate([p64, np.arange(64, HD)])

    cosT = cos.T.astype(np.float32)        # [64, T], cos[i] == cos[i+32]
    sinT = sin.T.astype(np.float32)
    cosP = cosT[p64]
    sinP = np.empty_like(cosP)
    sinP[0::2] = -sinT[:32]
    sinP[1::2] = sinT[:32]
    cs = np.ascontiguousarray(
        np.concatenate([cosP, sinP], axis=0)).astype(ml_dtypes.bfloat16)

    xTs = [np.ascontiguousarray(x[b].T).astype(ml_dtypes.bfloat16)
           for b in range(B)]
    in_maps = []
    for c in range(8):
        b, g = divmod(c, 4)
        wq_s = (wq[g * NH_L * HD:(g + 1) * NH_L * HD] * wn[None, :]).reshape(
            NH_L, HD, D)[:, perm, :].reshape(NH_L * HD, D)
        wk_s = (wk[g * HD:(g + 1) * HD] * wn[None, :])[perm]
        wv_s = wv[g * HD:(g + 1) * HD] * wn[None, :]
        wqkvT = np.ascontiguousarray(
            np.concatenate([wq_s, wk_s, wv_s], axis=0).T
        ).astype(ml_dtypes.bfloat16)                       # [D, 768]
        woT_s = np.ascontiguousarray(
            wo[:, g * NH_L * HD:(g + 1) * NH_L * HD].T
        ).astype(ml_dtypes.bfloat16)                       # [512, D]
        in_maps.append({
            "xT": xTs[b],
            "wqkvT": wqkvT,
            "woT": woT_s,
            "cs": cs,
        })
    return in_maps


def kernel(x, cos, sin, w_norm, wq, wk, wv, wo, rope_dims=64, use_xsa=1,
           **_unused):
    if "nc" not in _CACHE:
        _CACHE["nc"] = _build_nc()
    nc = _CACHE["nc"]
    in_maps = _host_inputs(
        np.asarray(x), np.asarray(cos), np.asarray(sin), np.asarray(w_norm),
        np.asarray(wq), np.asarray(wk), np.asarray(wv), np.asarray(wo),
    )
    res_obj = run_bass_kernel_spmd(nc, in_maps, list(range(8)))
    _CACHE["last"] = res_obj
    res = res_obj.results
    out = np.zeros((B, T, D), dtype=np.float32)
    for c in range(8):
        b = c // 4
        out[b] += np.asarray(res[c]["out"], dtype=np.float32)
    return out


# revision 22
# speedup vs baseline: 1.9952x; 1.0669x over previous
"""GQA attention block (RMSNorm + QKV proj + partial RoPE + causal GQA
attention + XSA correction + out proj) on 8 trn2 NeuronCores.

Sharding: 2 batches x 4 KV-groups (each core: 1 batch, 1 kv head, 4 q heads).
Each core computes a partial output (its 4 heads through its wo column slice);
the host sums the 4 partials per batch.

v2 design notes (all-bf16 dataflow):
 - host pre-casts x/wqkv/wo to bf16 and folds w_norm into the projection
   weights; no on-chip dtype casts at all.
 - rope rotate-half is turned into an adjacent-partition swap by permuting
   the first 64 rows of wq/wk (and the cos/sin tables) on the host, so the
   swap is a single DVE stream_shuffle (no sbuf-to-sbuf DMAs).
 - every reciprocal / rsqrt is computed as exp(-ln(x)) on the scalar engine;
   ln+exp live in one activation table set so there is no table thrashing.
 - causal masking uses partial-width score/sum/PV matmuls plus one
   gpsimd affine_select on the 128-wide diagonal block.
 - the PE instruction stream is ordered so QKV(j+1) / outproj(j-1) fill the
   windows where attention waits on rope/XSA, keeping the HAM clock gate
   warm (PE idle >3.4us re-throttles the PE clock 2x).
"""

import sys

for _p in ("/opt/trn_rl_repo", "/root/.axon_site/_ro/trn_rl_repo"):
    if _p not in sys.path:
        sys.path.append(_p)

import numpy as np
import ml_dtypes

import concourse.bass as bass
import concourse.bacc as bacc
import concourse.mybir as mybir
import concourse.tile as tile
from concourse import hw_specs as _hw_specs
from concourse.bass_utils import run_bass_kernel_spmd
from concourse.masks import make_identity

# The activation-table chooser maps Ln -> "natural_log" and Exp ->
# "exp_and_others", so a kernel alternating ln/exp reloads the table RAMs
# (~2.7us) on every switch.  Both functions live together in
# "natural_log_exp_and_others"; restrict them to that set so exactly one
# table load is ever emitted.
_ORIG_GAT = _hw_specs.get_activation_tables


def _gat_combined(arch):
    tabs = _ORIG_GAT(arch)
    keep = "natural_log_exp_and_others"
    if keep in tabs:
        ln = mybir.ActivationFunctionType.Ln
        ex = mybir.ActivationFunctionType.Exp
        for nm, fns in tabs.items():
            if nm != keep:
                fns.discard(ln)
                fns.discard(ex)
    return tabs


_hw_specs.get_activation_tables = _gat_combined
bacc.get_activation_tables = _gat_combined

F32 = mybir.dt.float32
BF16 = mybir.dt.bfloat16

B, T, D = 2, 2048, 2048
NH, NKV, HD = 16, 4, 128
RD = 64                    # rope dims
NH_L = NH // NKV           # 4 q heads per core
EL = (NH_L + 2) * HD       # 768: q0..q3, k, v columns
TC = 512                   # token chunk
NTC = T // TC              # 4
DC = D // 128              # 16 contraction chunks
S128 = float(1.0 / np.sqrt(HD))
EPS = 1e-6

# DVE stream_shuffle mask: swap adjacent partitions within each 32-quadrant
SWAP_MASK = []
for _i in range(16):
    SWAP_MASK += [2 * _i + 1, 2 * _i]

_CACHE = {}


def _build_nc():
    nc = bacc.Bacc("TRN2", target_bir_lowering=False, debug=False)

    xT = nc.declare_dram_parameter("xT", [D, T], BF16, isOutput=False)
    wT = nc.declare_dram_parameter("wqkvT", [D, EL], BF16, isOutput=False)
    woT = nc.declare_dram_parameter("woT", [NH_L * HD, D], BF16, isOutput=False)
    csP = nc.declare_dram_parameter("cs", [128, T], BF16, isOutput=False)
    outp = nc.declare_dram_parameter("out", [T, D], F32, isOutput=True)

    ACT = mybir.ActivationFunctionType

    with tile.TileContext(nc) as tc:
        with (
            nc.allow_low_precision(reason="bf16 dataflow; rel tol 2e-2"),
            tc.tile_pool(name="singles", bufs=1) as sg,
            tc.tile_pool(name="stream", bufs=2) as st,
            tc.tile_pool(name="ps", bufs=1, space="PSUM") as ps,
        ):
            # ---------------- persistent tiles ----------------
            w_sb = sg.tile([128, DC * EL], BF16, tag="w")
            wo_sb = sg.tile([128, 16 * TC], BF16, tag="wo")
            cosP = sg.tile([RD, T], BF16, tag="cosP")
            sinP = sg.tile([RD, T], BF16, tag="sinP")
            ident = sg.tile([128, 128], BF16, tag="ident")
            ones_cb = sg.tile([128, 1], BF16, tag="ones_cb")
            eps_t = sg.tile([1, 1], F32, tag="eps_t")
            kh = [sg.tile([128, TC], BF16, tag=f"kh{j}", name=f"kh{j}")
                  for j in range(NTC)]
            vh = [sg.tile([128, TC], BF16, tag=f"vh{j}", name=f"vh{j}")
                  for j in range(NTC)]
            vt = [sg.tile([128, TC], BF16, tag=f"vt{j}", name=f"vt{j}")
                  for j in range(NTC)]
            rvns = [sg.tile([1, TC], F32, tag=f"rvns{j}", name=f"rvns{j}")
                    for j in range(NTC)]

            # x tiles live in a rotating 3-chunk window (SBUF pressure);
            # chunk j's DMAs reuse chunk j-3's buffers once qkv(j-3) is done.
            xtl = {}

            def xsl(j, i):
                return xtl[(j, i)]

            def emit_xload(j):
                js = slice(j * TC, (j + 1) * TC)
                for i in range(DC):
                    xtl[(j, i)] = st.tile([128, TC], BF16, tag="xt",
                                          bufs=3 * DC, name=f"x_{j}_{i}")
                    nc.sync.dma_start(
                        out=xtl[(j, i)], in_=xT[i * 128:(i + 1) * 128, js])

            # ---------------- initial DMAs ----------------
            nc.sync.dma_start(out=cosP, in_=csP[0:RD, :])
            nc.sync.dma_start(out=sinP, in_=csP[RD:128, :])
            # interleave k/v weight columns with x chunk 0 so the i-th
            # pass-B matmul can start as soon as its own tiles land
            for i in range(DC):
                nc.sync.dma_start(
                    out=w_sb[:, i * EL + 4 * HD:(i + 1) * EL],
                    in_=wT[i * 128:(i + 1) * 128, 4 * HD:EL],
                )
                xtl[(0, i)] = st.tile([128, TC], BF16, tag="xt",
                                      bufs=3 * DC, name=f"x_0_{i}")
                nc.sync.dma_start(
                    out=xtl[(0, i)], in_=xT[i * 128:(i + 1) * 128, 0:TC])
            # q weight columns
            for i in range(DC):
                nc.sync.dma_start(
                    out=w_sb[:, i * EL:i * EL + 4 * HD],
                    in_=wT[i * 128:(i + 1) * 128, 0:4 * HD],
                )
            emit_xload(1)
            emit_xload(2)
            # wo tiles: (h, m) at column (h*4+m)*TC
            for h in range(NH_L):
                for m in range(4):
                    nc.sync.dma_start(
                        out=wo_sb[:, (h * 4 + m) * TC:(h * 4 + m + 1) * TC],
                        in_=woT[h * 128:(h + 1) * 128, m * TC:(m + 1) * TC],
                    )
            # chunk 3's x loads wait on chunk 0's buffers; issue them after wo
            # so the in-order sync queue doesn't hold the wo transfers back.
            emit_xload(3)

            make_identity(nc, ident)
            nc.vector.memset(ones_cb, 1.0)
            nc.vector.memset(eps_t, EPS)

            # x^2 tiles for the rms-norm sum (created one chunk ahead)
            x2 = {}
            state = {}

            def emit_x2(j):
                for i in range(DC):
                    x2[(j, i)] = st.tile([128, TC], BF16, tag="x2",
                                         bufs=DC, name=f"x2_{j}_{i}")
                    nc.vector.tensor_mul(x2[(j, i)], xsl(j, i), xsl(j, i))

            def emit_qkvB(j):
                # ---- pass B: k, v, rs sums on PE ----
                ps_k = ps.tile([128, TC], F32, tag="big", bufs=6, name=f"psk{j}")
                ps_v = ps.tile([128, TC], F32, tag="big", bufs=6, name=f"psv{j}")
                sm_rs = ps.tile([1, TC], F32, tag="sm", bufs=2, name=f"smrs{j}")
                for i in range(DC):
                    wof = i * EL
                    nc.tensor.matmul(
                        ps_k, w_sb[:, wof + 4 * HD:wof + 5 * HD], xsl(j, i),
                        start=(i == 0), stop=(i == DC - 1))
                    nc.tensor.matmul(
                        ps_v, w_sb[:, wof + 5 * HD:wof + 6 * HD], xsl(j, i),
                        start=(i == 0), stop=(i == DC - 1))
                    nc.tensor.matmul(
                        sm_rs, ones_cb, x2[(j, i)],
                        start=(i == 0), stop=(i == DC - 1))
                # rs = exp(-0.5*ln(mean(x^2)+eps))  (scalar engine only)
                lnr = st.tile([1, TC], F32, tag="lnr", bufs=2)
                nc.scalar.activation(lnr, sm_rs, ACT.Ln, scale=1.0 / D,
                                     bias=eps_t)
                rs_t = st.tile([1, TC], BF16, tag="rs_t", bufs=2)
                nc.scalar.activation(rs_t, lnr, ACT.Exp, scale=-0.5)
                rsb = st.tile([128, TC], BF16, tag="rsb", bufs=2)
                nc.gpsimd.partition_broadcast(rsb, rs_t)
                state[("rsb", j)] = (ps_k, ps_v, rsb)

            def emit_qkvA(j):
                js = slice(j * TC, (j + 1) * TC)
                ps_k, ps_v, rsb = state[("rsb", j)]
                ps_q = [ps.tile([128, TC], F32, tag="big", bufs=6,
                                name=f"psq{j}_{h}") for h in range(NH_L)]
                for i in range(DC):
                    wof = i * EL
                    for h in range(NH_L):
                        nc.tensor.matmul(
                            ps_q[h], w_sb[:, wof + h * HD:wof + (h + 1) * HD],
                            xsl(j, i), start=(i == 0), stop=(i == DC - 1))

                def rope(th):
                    t2s = st.tile([RD, TC], BF16, tag="t2s", bufs=2)
                    nc.vector.stream_shuffle(t2s, th[0:RD], SWAP_MASK)
                    nc.vector.tensor_mul(th[0:RD], th[0:RD], cosP[:, js])
                    nc.vector.tensor_mul(t2s, t2s, sinP[:, js])
                    nc.vector.tensor_add(th[0:RD], th[0:RD], t2s)

                # evacuate with rms scale folded in (DVE); k/q0/q1 and their
                # ropes first so attnA(j) can start as early as possible
                qhj = [
                    st.tile([128, TC], BF16, tag="qh", bufs=8,
                            name=f"qh{j}_{h}")
                    for h in range(NH_L)
                ]
                nc.vector.tensor_mul(kh[j], ps_k, rsb)
                nc.vector.tensor_mul(qhj[0], ps_q[0], rsb)
                nc.vector.tensor_mul(qhj[1], ps_q[1], rsb)
                rope(kh[j])
                rope(qhj[0])
                rope(qhj[1])
                nc.vector.tensor_mul(vh[j], ps_v, rsb)
                vsq = st.tile([128, TC], BF16, tag="vsq", bufs=2,
                              name=f"vsq{j}")
                nc.vector.tensor_mul(vsq, vh[j], vh[j])
                nc.vector.tensor_mul(qhj[2], ps_q[2], rsb)
                nc.vector.tensor_mul(qhj[3], ps_q[3], rsb)
                rope(qhj[2])
                rope(qhj[3])
                state[("vsq", j)] = vsq
                state[("qh", j)] = qhj

            def emit_vt(j):
                # v token-major transpose (PE), own phase so the in-order PE
                # queue never waits on vh here
                ps_vt = ps.tile([128, TC], BF16, tag="big", bufs=6,
                                name=f"psvt{j}")
                for kk in range(TC // 128):
                    nc.tensor.transpose(
                        ps_vt[:, kk * 128:(kk + 1) * 128],
                        vh[j][:, kk * 128:(kk + 1) * 128],
                        ident,
                    )
                nc.scalar.copy(vt[j], ps_vt)

            def emit_vns(j):
                vsq = state[("vsq", j)]
                sm_vns = ps.tile([1, TC], F32, tag="sm", bufs=2,
                                 name=f"smvns{j}")
                nc.tensor.matmul(sm_vns, ones_cb, vsq, start=True, stop=True)
                lnv = st.tile([1, TC], F32, tag="lnv", bufs=2)
                nc.scalar.activation(lnv, sm_vns, ACT.Ln, scale=1.0,
                                     bias=eps_t)
                nc.scalar.activation(rvns[j], lnv, ACT.Exp, scale=-1.0)

            def emit_attn_pair(j, pair):
                heads = (0, 1) if pair == 0 else (2, 3)
                qhj = state[("qh", j)]
                nkt = 4 * (j + 1)
                ps_pv = {
                    h: ps.tile([128, TC], F32, tag="big", bufs=6,
                               name=f"pspv{j}_{h}")
                    for h in heads
                }
                sm_sum = ps.tile([64, TC], F32, tag="sm", bufs=2,
                                 name=f"smsum{j}_{pair}")
                pT = {}

                def tile_geom(kt):
                    cs = 128 * (kt - 4 * j) if kt >= 4 * j else 0
                    return cs, TC - cs

                def emit_sum_pv(kt):
                    cs, _w = tile_geom(kt)
                    jk, kk = divmod(kt, 4)
                    for h in heads:
                        p = pT[(h, kt)][:, cs:TC]
                        nc.tensor.matmul(
                            sm_sum[32 * (h % 2):32 * (h % 2) + 1, cs:TC],
                            ones_cb, p,
                            start=(kt == 0), stop=(kt == nkt - 1))
                        nc.tensor.matmul(
                            ps_pv[h][:, cs:TC],
                            vt[jk][:, kk * 128:(kk + 1) * 128], p,
                            start=(kt == 0), stop=(kt == nkt - 1))

                # sum/pv for tile kt-2 issue while kt's scores compute: by
                # then exp(kt-2) is long done, so the PE's LDWEIGHTS
                # prefetch is never blocked on a pending semaphore.
                for kt in range(nkt):
                    cs, _w = tile_geom(kt)
                    jk, kk = divmod(kt, 4)
                    for h in heads:
                        ps_sc = ps.tile([128, TC], F32, tag="big", bufs=6,
                                        name=f"pssc{j}_{h}_{kt}")
                        nc.tensor.matmul(
                            ps_sc[:, cs:TC],
                            kh[jk][:, kk * 128:(kk + 1) * 128],
                            qhj[h][:, cs:TC],
                            start=True, stop=True)
                        pt = st.tile([128, TC], BF16, tag="pT", bufs=8,
                                     name=f"pt{j}_{h}_{kt}")
                        nc.scalar.activation(
                            pt[:, cs:TC], ps_sc[:, cs:TC], ACT.Exp, scale=S128)
                        if kt >= 4 * j:
                            # zero strictly-above-diagonal entries in the
                            # 128-wide diagonal block: keep col >= partition
                            nc.gpsimd.affine_select(
                                out=pt[:, cs:cs + 128], in_=pt[:, cs:cs + 128],
                                compare_op=mybir.AluOpType.is_ge,
                                fill=0.0, base=0,
                                pattern=[[1, 128]],
                                channel_multiplier=-1,
                            )
                        pT[(h, kt)] = pt
                    if kt > 1:
                        emit_sum_pv(kt - 2)
                if nkt > 1:
                    emit_sum_pv(nkt - 2)
                emit_sum_pv(nkt - 1)
                state[(j, pair)] = (ps_pv, sm_sum)

            def emit_xsa_pre(j, pair):
                heads = (0, 1) if pair == 0 else (2, 3)
                ps_pv, sm_sum = state[(j, pair)]
                pvsb, tu, inv = {}, {}, {}
                for h in heads:
                    pvsb[h] = st.tile([128, TC], BF16, tag="pvsb", bufs=4,
                                      name=f"pvsb{j}_{h}")
                    nc.scalar.copy(pvsb[h], ps_pv[h])
                    lnS = st.tile([1, TC], F32, tag="lnS", bufs=2)
                    nc.scalar.activation(
                        lnS, sm_sum[32 * (h % 2):32 * (h % 2) + 1, :], ACT.Ln)
                    inv[h] = st.tile([1, TC], BF16, tag="inv", bufs=4,
                                     name=f"inv{j}_{h}")
                    nc.scalar.activation(inv[h], lnS, ACT.Exp, scale=-1.0)
                    tu[h] = st.tile([128, TC], BF16, tag="tu", bufs=4,
                                    name=f"tu{j}_{h}")
                    nc.vector.tensor_mul(tu[h], pvsb[h], vh[j])
                state[(j, pair, "pre")] = (pvsb, tu, inv)

            def emit_dots(j, pair):
                heads = (0, 1) if pair == 0 else (2, 3)
                _, tu, _ = state[(j, pair, "pre")]
                sm_dot = ps.tile([64, TC], F32, tag="sm", bufs=2,
                                 name=f"smdot{j}_{pair}")
                for h in heads:
                    nc.tensor.matmul(
                        sm_dot[32 * (h % 2):32 * (h % 2) + 1, :],
                        ones_cb, tu[h], start=True, stop=True)
                state[(j, pair, "dot")] = sm_dot

            def emit_xsa_post(j, pair):
                heads = (0, 1) if pair == 0 else (2, 3)
                pvsb, tu, inv = state[(j, pair, "pre")]
                sm_dot = state[(j, pair, "dot")]
                for h in heads:
                    t1 = st.tile([1, TC], F32, tag="t1", bufs=2)
                    nc.vector.tensor_mul(
                        t1, sm_dot[32 * (h % 2):32 * (h % 2) + 1, :], rvns[j])
                    fui = st.tile([1, TC], BF16, tag="fui", bufs=2)
                    nc.vector.tensor_mul(fui, t1, inv[h])
                    invB = st.tile([128, TC], BF16, tag="invB", bufs=2)
                    nc.gpsimd.partition_broadcast(invB, inv[h])
                    fuiB = st.tile([128, TC], BF16, tag="fuiB", bufs=2)
                    nc.gpsimd.partition_broadcast(fuiB, fui)
                    m1 = st.tile([128, TC], BF16, tag="m1", bufs=2)
                    nc.vector.tensor_mul(m1, pvsb[h], invB)
                    m2 = st.tile([128, TC], BF16, tag="m2", bufs=2)
                    nc.vector.tensor_mul(m2, vh[j], fuiB)
                    aot = st.tile([128, TC], BF16, tag="ao", bufs=6,
                                  name=f"ao{j}_{h}")
                    nc.vector.tensor_sub(aot, m1, m2)
                    state[("ao", j, h)] = aot

            def emit_outproj(j):
                for m in range(4):
                    ms = slice(m * TC, (m + 1) * TC)
                    for tt in range(4):
                        ps_o = ps.tile([128, TC], F32, tag="big", bufs=6,
                                       name=f"pso{j}_{m}_{tt}")
                        for h in range(NH_L):
                            nc.tensor.matmul(
                                ps_o,
                                state[("ao", j, h)][:, tt * 128:(tt + 1) * 128],
                                wo_sb[:, (h * 4 + m) * TC:(h * 4 + m + 1) * TC],
                                start=(h == 0), stop=(h == NH_L - 1))
                        osb = st.tile([128, TC], F32, tag="osb", bufs=3)
                        if tt % 2 == 0:
                            nc.scalar.copy(osb, ps_o)
                        else:
                            nc.vector.tensor_copy(osb, ps_o)
                        nc.sync.dma_start(
                            out=outp[(j * 4 + tt) * 128:(j * 4 + tt + 1) * 128,
                                     ms],
                            in_=osb)

            # ================= the schedule =================
            # PE stream: QKV(0) | attnA(0) attnB(0) dotsA(0) | QKV(1) dotsB(0)
            #            outproj(0) | attnA(1) ... so rope(j+1)/XSA(j) on the
            #            other engines always overlap PE matmul phases.
            # PE stream per j: attnA attnB vns dotsA | qkvB(j+1) dotsB
            # qkvA(j+1) | outproj(j) vt(j+1) | attnA(j+1) ... ; the qkv
            # passes cover the XSA/rope latencies on scalar/DVE/gpsimd.
            emit_x2(0)
            emit_qkvB(0)
            emit_qkvA(0)
            emit_vt(0)
            for j in range(NTC):
                emit_attn_pair(j, 0)
                emit_xsa_pre(j, 0)
                emit_attn_pair(j, 1)
                emit_xsa_pre(j, 1)
                emit_vns(j)
                emit_dots(j, 0)
                emit_xsa_post(j, 0)
                if j + 1 < NTC:
                    emit_x2(j + 1)
                    emit_qkvB(j + 1)
                emit_dots(j, 1)
                emit_xsa_post(j, 1)
                if j + 1 < NTC:
                    emit_qkvA(j + 1)
                emit_outproj(j)
                if j + 1 < NTC:
                    emit_vt(j + 1)

    nc.compile()
    return nc


def _host_inputs(x, cos, sin, w_norm, wq, wk, wv, wo):
    """Build the 8 per-core input maps (host-side layout prep only)."""
    wn = w_norm.astype(np.float32)
    # rope interleave permutation: pair (i, i+32) -> positions (2i, 2i+1)
    p64 = np.empty(64, np.int64)
    p64[0::2] = np.arange(32)
    p64[1::2] = np.arange(32, 64)
    perm = np.concatenate([p64, np.arange(64, HD)])

    cosT = cos.T.astype(np.float32)        # [64, T], cos[i] == cos[i+32]
    sinT = sin.T.astype(np.float32)
    cosP = cosT[p64]
    sinP = np.empty_like(cosP)
    sinP[0::2] = -sinT[:32]
    sinP[1::2] = sinT[:32]
    cs = np.ascontiguousarray(
        np.concatenate([cosP, sinP], axis=0)).astype(ml_dtypes.bfloat16)

    xTs = [np.ascontiguousarray(x[b].T).astype(ml_dtypes.bfloat16)
           for b in range(B)]
    in_maps = []
    for c in range(8):
        b, g = divmod(c, 4)
        wq_s = (wq[g * NH_L * HD:(g + 1) * NH_L * HD] * wn[None, :]).reshape(
            NH_L, HD, D)[:, perm, :].reshape(NH_L * HD, D)
        wk_s = (wk[g * HD:(g + 1) * HD] * wn[None, :])[perm]
        wv_s = wv[g * HD:(g + 1) * HD] * wn[None, :]
        wqkvT = np.ascontiguousarray(
            np.concatenate([wq_s, wk_s, wv_s], axis=0).T
        ).astype(ml_dtypes.bfloat16)                       # [D, 768]
        woT_s = np.ascontiguousarray(
            wo[:, g * NH_L * HD:(g + 1) * NH_L * HD].T
        ).astype(ml_dtypes.bfloat16)                       # [512, D]
        in_maps.append({
            "xT": xTs[b],
            "wqkvT": wqkvT,
            "woT": woT_s,
            "cs": cs,
        })
    return in_maps


def kernel(x, cos, sin, w_norm, wq, wk, wv, wo, rope_dims=64, use_xsa=1,
           **_unused):
    if "nc" not in _CACHE:
        _CACHE["nc"] = _build_nc()
    nc = _CACHE["nc"]
    in_maps = _host_inputs(
        np.asarray(x), np.asarray(cos), np.asarray(sin), np.asarray(w_norm),
        np.asarray(wq), np.asarray(wk), np.asarray(wv), np.asarray(wo),
    )
    res_obj = run_bass_kernel_spmd(nc, in_maps, list(range(8)))
    _CACHE["last"] = res_obj
    res = res_obj.results
    out = np.zeros((B, T, D), dtype=np.float32)
    for c in range(8):
        b = c // 4
        out[b] += np.asarray(res[c]["out"], dtype=np.float32)
    return out
